# revision 1
# baseline (speedup 1.0000x reference)
"""Trainium2 Bass kernel: inclusive cumsum along L for X (4, 8192, 32, 32) f32.

Strategy (8 NeuronCores, SPMD): bf16 traffic + all-matmul Blelloch scan.
  - Shard: core i gets b = i//2, c-half = i%2 -> a (8192, 512) slab, cast to
    bf16 on the host (HBM per core: 8 MiB in + 8 MiB out, ~47 us roofline vs
    ~94 us for the f32 baseline). The host also pre-arranges each slab
    superblock-major [8, 128, 8*512] so every 1 MiB DMA is fully contiguous
    with 8 KiB per-partition runs (1 KiB runs are descriptor-count bound at
    ~half bandwidth). Only 16 DMAs total; X stays fully SBUF-resident.
  - The scan runs entirely on the TensorEngine (matmul computes lhsT.T@rhs;
    DVE tensor_tensor_scan is ~4x too slow at f32, and cross-partition work
    is PE-only):
    phase 1: per group of 16 row-blocks, one-hot-column stationaries
      accumulate block column-sums S[16, 512] into one PSUM bank;
    phase 2: 4 small matmuls compute T[16, 512] = carry + exclusive
      prefix(S) and the next carry [1, 512] (engine APs must start at
      partition 0/32/64/96, so the running carry lives at partition 0);
    phase 3: per block, a row-selector matmul broadcasts T_i onto all 128
      partitions of a PSUM bank (start=True) and the inclusive
      upper-triangular matmul accumulates the within-block prefix on top;
      ScalarE/DVE (alternating) copy PSUM -> bf16 output tiles.
  - The PE clock-gate (HAM) holds non-dense matmul streams at 4/8 = 1.2 GHz
    on this part (535 ns per 512-col bf16 matmul vs 268 warm); the stream
    here is dense enough to run partially warm. Issuing phase-3 in clusters
    of 4 (4 carry matmuls, then 4 UT matmuls sharing one stationary load)
    cut the mean core time from ~103 to ~87 us. Measured: 87 us mean /
    94 us slowest-core (baseline 112-115 us f32; DMA-broadcast carry
    variants measured 172-314 us due to ~6 us/hop DMA latency and SBUF
    port limits, and are strictly worse).
  - Error budget (tolerance 2e-2 * max|out| ~ 9.1): bf16 input quantization
    random-walks to ~0.3 abs; bf16 carry chain across 3 group boundaries
    ~2.7 worst-case; T/output roundings ~0.9 each. Measured ~3.6 abs
    (7.8e-3 relative), a 2.5x margin.
"""

import numpy as np
import ml_dtypes
from contextlib import ExitStack

import concourse.bass as bass
import concourse.tile as tile
from concourse import bacc, masks, mybir
from concourse.bass_utils import run_bass_kernel_spmd

N_CORES = 8
B, L, D, N = 4, 8192, 32, 32
C_FULL = D * N
C = C_FULL // 2
P = 128
NBLK = L // P
GBLK = 16
NGRP = NBLK // GBLK
SBB = 8
NSB = NBLK // SBB
SBW = SBB * C

_CACHE = {}


def _build_program():
    f32 = mybir.dt.float32
    bf16 = mybir.dt.bfloat16
    nc = bacc.Bacc(
        trn_type="TRN2", debug=False, num_devices=N_CORES, num_swdge_queues=2
    )
    x = nc.dram_tensor("x", [NSB, P, SBW], bf16, kind="ExternalInput").ap()
    y = nc.dram_tensor("y", [NSB, P, SBW], bf16, kind="ExternalOutput").ap()

    with tile.TileContext(nc) as tc, ExitStack() as ctx:
        const_pool = ctx.enter_context(tc.tile_pool(name="const", bufs=1))
        xin_pool = ctx.enter_context(tc.tile_pool(name="xin", bufs=1))
        yout_pool = ctx.enter_context(tc.tile_pool(name="yout", bufs=6))
        small_pool = ctx.enter_context(tc.tile_pool(name="small", bufs=2))
        yps_pool = ctx.enter_context(tc.tile_pool(name="yps", bufs=5, space="PSUM"))
        sps_pool = ctx.enter_context(tc.tile_pool(name="sps", bufs=1, space="PSUM"))
        tps_pool = ctx.enter_context(tc.tile_pool(name="tps", bufs=1, space="PSUM"))

        ut = const_pool.tile([P, P], bf16, name="ut")
        masks.make_upper_triangular(nc, ut[:], 1.0, diag=True)
        # Z1Z: ones in column GBLK-1; a 16-wide slice puts the ones-column
        # at any position 0..15 (phase-1 one-hot stationaries).
        z1z = const_pool.tile([P, 2 * GBLK - 1], bf16, name="z1z")
        nc.gpsimd.memset(z1z[:], 0.0)
        nc.gpsimd.memset(z1z[:, GBLK - 1 : GBLK], 1.0)
        # RZ row-selector bank: slice [:, i*128:(i+1)*128] is all-ones in
        # row i -> matmul replicates T row i onto all 128 output partitions.
        rz = const_pool.tile([GBLK, GBLK * P], bf16, name="rz")
        nc.gpsimd.memset(rz[:], 1.0)
        nc.gpsimd.affine_select(
            out=rz[:], in_=rz[:], compare_op=mybir.AluOpType.is_ge,
            fill=0.0, base=0, pattern=[[1, GBLK * P]], channel_multiplier=-P,
        )
        nc.gpsimd.affine_select(
            out=rz[:], in_=rz[:], compare_op=mybir.AluOpType.is_ge,
            fill=0.0, base=P - 1, pattern=[[-1, GBLK * P]], channel_multiplier=P,
        )
        tms = const_pool.tile([GBLK, GBLK], bf16, name="tms")
        masks.make_upper_triangular(nc, tms[:], 1.0, diag=False)
        ones_1x16 = const_pool.tile([1, GBLK], bf16, name="ones_1x16")
        nc.gpsimd.memset(ones_1x16[:], 1.0)
        ones_16x1 = const_pool.tile([GBLK, 1], bf16, name="ones_16x1")
        nc.gpsimd.memset(ones_16x1[:], 1.0)
        one_1x1 = const_pool.tile([1, 1], bf16, name="one_1x1")
        nc.gpsimd.memset(one_1x1[:], 1.0)
        ca0 = const_pool.tile([1, C], bf16, name="ca0")
        nc.gpsimd.memset(ca0[:], 0.0)

        prev_ca = ca0
        xts = {}

        # issue every in-DMA upfront: X stays fully SBUF-resident (64 KiB of
        # 208 per partition), so once loaded the PE never waits on input and
        # the HAM clock-gate can hold at 8/8.
        for s in range(NSB):
            xt = xin_pool.tile([P, SBW], bf16, name=f"xt{s}", tag=f"xt{s}", bufs=1)
            (nc.sync if s % 2 == 0 else nc.scalar).dma_start(out=xt[:], in_=x[s])
            xts[s] = xt

        def emit_phase1(g):
            sp = sps_pool.tile([GBLK, C], f32, name="sp", tag="sp", bufs=1)
            for i in range(GBLK):
                blk = GBLK * g + i
                s, k = blk // SBB, blk % SBB
                nc.tensor.matmul(
                    sp[:],
                    z1z[:, GBLK - 1 - i : 2 * GBLK - 1 - i],
                    xts[s][:, k * C : (k + 1) * C],
                    start=(i == 0),
                    stop=(i == GBLK - 1),
                )
            # drain S to SBUF right away so the single S-PSUM bank frees
            sa = small_pool.tile([GBLK, C], bf16, name="sa", tag="sa", bufs=2)
            nc.vector.tensor_copy(sa[:], sp[:])
            return sa

        def emit_carry_math(g, sa):
            nonlocal prev_ca
            ca = prev_ca
            tp = tps_pool.tile([GBLK, C], f32, name="tp", tag="tp", bufs=1)
            nc.tensor.matmul(tp[:], ones_1x16[:], ca[:], start=True, stop=False)
            nc.tensor.matmul(tp[:], tms[:], sa[:], start=False, stop=True)
            tb = small_pool.tile([GBLK, C], bf16, name="tb", tag="tb", bufs=2)
            nc.vector.tensor_copy(tb[:], tp[:])
            if g < NGRP - 1:
                cp = tps_pool.tile([1, C], f32, name="cp", tag="cp", bufs=1)
                nc.tensor.matmul(cp[:], ones_16x1[:], sa[:], start=True, stop=False)
                nc.tensor.matmul(cp[:], one_1x1[:], ca[:], start=False, stop=True)
                nca = small_pool.tile([1, C], bf16, name="nca", tag="nca", bufs=2)
                nc.vector.tensor_copy(nca[:], cp[:])
                prev_ca = nca
            return tb

        def emit_phase3(g, tb):
            yt = None
            # clusters of 4: rz x4 then ut x4 so the UT stationary is loaded
            # once per four blocks instead of every block.
            for c0 in range(0, GBLK, 4):
                pend = []
                for i in range(c0, c0 + 4):
                    blk = GBLK * g + i
                    s, k = blk // SBB, blk % SBB
                    if k == 0:
                        yt = yout_pool.tile(
                            [P, SBW], bf16, name=f"yt{s}", tag="yt", bufs=6
                        )
                    yp = yps_pool.tile([P, C], f32, name="yp", tag="yp", bufs=5)
                    nc.tensor.matmul(
                        yp[:], rz[:, i * P : (i + 1) * P], tb[:],
                        start=True, stop=False,
                    )
                    pend.append((i, yp, yt))
                for i, yp, yti in pend:
                    blk = GBLK * g + i
                    s, k = blk // SBB, blk % SBB
                    nc.tensor.matmul(
                        yp[:], ut[:], xts[s][:, k * C : (k + 1) * C],
                        start=False, stop=True,
                    )
                    if blk % 2 == 1:
                        nc.vector.tensor_copy(yti[:, k * C : (k + 1) * C], yp[:])
                    else:
                        nc.scalar.copy(yti[:, k * C : (k + 1) * C], yp[:])
                    if k == SBB - 1:
                        (nc.scalar if s % 2 == 0 else nc.sync).dma_start(
                            out=y[s], in_=yti[:]
                        )

        # schedule: ph_0, ph_1, T_0, p3_0, ph_2, T_1, p3_1, ph_3, T_2, p3_2, T_3, p3_3
        sas = {}
        tbs = {}
        sas[0] = emit_phase1(0)
        sas[1] = emit_phase1(1)
        tbs[0] = emit_carry_math(0, sas[0])
        emit_phase3(0, tbs[0])
        sas[2] = emit_phase1(2)
        tbs[1] = emit_carry_math(1, sas[1])
        emit_phase3(1, tbs[1])
        sas[3] = emit_phase1(3)
        tbs[2] = emit_carry_math(2, sas[2])
        emit_phase3(2, tbs[2])
        tbs[3] = emit_carry_math(3, sas[3])
        emit_phase3(3, tbs[3])

    nc.compile()
    return nc


def _get_program():
    if "nc" not in _CACHE:
        _CACHE["nc"] = _build_program()
    return _CACHE["nc"]


def _shard(X):
    Xv = X.reshape(B, L, C_FULL)
    shards = []
    for i in range(N_CORES):
        b, h = i // 2, i % 2
        slab = Xv[b, :, h * C : (h + 1) * C]
        arr = (
            slab.reshape(NSB, SBB, P, C).transpose(0, 2, 1, 3).reshape(NSB, P, SBW)
        )
        shards.append(np.ascontiguousarray(arr).astype(ml_dtypes.bfloat16))
    return shards


def _unshard(parts):
    out = np.empty((B, L, C_FULL), dtype=np.float32)
    for i in range(N_CORES):
        b, h = i // 2, i % 2
        arr = np.asarray(parts[i]).astype(np.float32)
        slab = arr.reshape(NSB, P, SBB, C).transpose(0, 2, 1, 3).reshape(L, C)
        out[b, :, h * C : (h + 1) * C] = slab
    return out.reshape(B, L, D, N)


def kernel(X_in, _trace=False, _tmpdir=None, _trace_cores=None):
    X = np.asarray(X_in, dtype=np.float32)
    assert X.shape == (B, L, D, N), X.shape
    nc = _get_program()
    in_maps = [{"x": s} for s in _shard(X)]
    kwargs = {}
    if _trace:
        kwargs = dict(
            trace=True,
            tmpdir=_tmpdir,
            trace_cores=_trace_cores or list(range(N_CORES)),
        )
    res = run_bass_kernel_spmd(nc, in_maps, core_ids=list(range(N_CORES)), **kwargs)
    out = _unshard([res.results[i]["y"] for i in range(N_CORES)])
    kernel.last_results = res
    return out



# revision 8
# speedup vs baseline: 1.3213x; 1.3213x over previous
"""Trainium2 Bass kernel: inclusive cumsum along L for X (4, 8192, 32, 32) f32.

Hybrid PE+DVE design (8 NeuronCores, SPMD), bf16 HBM traffic both ways:
  - Shard: core i gets b = i//2, channel-half = i%2 -> 512 channels x 8192 L,
    split 128 (PE pipeline) / 384 (DVE pipeline) so every engine sits under
    the ~40 us DMA roofline (16.8 MiB per core at ~420 GB/s aggregate).
  - DVE half (384 ch): host pre-transposes to [3][128ch][8192L] bf16. A
    custom DVE op (ANT_CUMSUM_INIT, registered per-NEFF: body =
    scan(ADD, Src0, init=C0)) computes the inclusive prefix along the free
    dim at 1 elem/cycle (~1.06 ns/elem measured, 2x the stock
    tensor_tensor_scan whose feedback bubble costs 2 cycles/elem) with fp32
    ALU state. 12 chunk-scans of [128, 2048]; chunks chain through an f32
    [P,1] state column (imm0 scalar APs must be f32).
  - PE half (128 ch): all-matmul Blelloch scan in L-on-partitions layout
    (superblock-major [8][128][8*128]): phase 1 accumulates per-group block
    column-sums S[16, 128] (one-hot stationaries); phase 2 computes
    T = carry + exclusive prefix(S); phase 3 processes QUADS of adjacent
    blocks per [128, 512] PSUM bank: 4 row-selector matmuls broadcast
    T_i onto the quarters (first start=True clears the bank's has_written
    bits; the rest start=False overwrite their still-clear quarters), then
    ONE 512-col upper-triangular matmul accumulates all 4 within-block
    prefixes. ScalarE drains each bank (16 copies of [128, 512]).
  - DMA rings: sync = PE ins + DVE tile-0 ins + DVE tile-0 outs (6.3 MiB);
    scalar = DVE tile-1/2 ins + PE outs (6.3 MiB); gpsimd SWDGE = DVE
    tile-1/2 outs (4.2 MiB). All ins first, interleaved by consumption
    order.
  - Error budget (tolerance 2e-2 * max|out| ~ 9.1): bf16 input quantization
    ~0.3; PE-half bf16 carry chain ~2.7 worst-case; DVE-half bf16 chunk
    chaining ~2.7 worst-case; output roundings ~1.8. Measured ~3.6 abs
    (7.8e-3 relative).
"""

import numpy as np
import ml_dtypes
from contextlib import ExitStack

import concourse.bass as bass
import concourse.tile as tile
from concourse import bacc, masks, mybir
from concourse.bass_utils import run_bass_kernel_spmd

N_CORES = 8
B, L, D, N = 4, 8192, 32, 32
C_FULL = D * N          # 1024 channels total
CH = C_FULL // 2        # 512 channels per core
C = 128                 # PE-half channels
CV = CH - C             # DVE-half channels (384)
P = 128
NBLK = L // P           # 64 blocks of 128 rows
GBLK = 16               # blocks per carry group
NGRP = NBLK // GBLK     # 4 groups
SBB = 8                 # blocks per superblock (DMA unit)
NSB = NBLK // SBB       # 8 superblocks
SBW = SBB * C           # 1024 cols per superblock tile
NVT = CV // P           # 3 DVE tiles of 128 channels
VCH = 2048              # DVE chunk width (cols of L)
NVC = L // VCH          # 4 chunks per DVE tile

_CACHE = {}


def _register_cumsum_op():
    """Per-NEFF custom DVE op: out[p,k] = s0[p] + sum_{j<=k} in0[p,j].
    Registered through the documented dve_ops extension point (appended to
    OPS with a computed uops_sha); runs at 1 elem/cycle vs the stock
    tensor_tensor_scan's 2 cycles/elem."""
    from concourse import dve_ops
    from concourse.dve_spec import Spec, Src0, C0, AluOp, scan, lower
    from concourse.dve_uop import DveOpSpec

    name = "ANT_CUMSUM_INIT"
    for op in dve_ops.OPS:
        if op.name == name:
            return op
    spec = Spec(
        body=scan(AluOp.ADD, Src0, init=C0),
        reference=lambda in0, s0: np.cumsum(in0.astype(np.float32), axis=-1)
        + np.asarray(s0, dtype=np.float32),
    )
    row = dve_ops._CUSTOM_DVE_ROW_BASE + len(dve_ops.OPS)
    sha = {}
    for ver in ("v3", "v4"):
        s = DveOpSpec(name=name, opcode=row, uops=lower(spec, ver=ver), rd1_en=False)
        sha[ver] = s.sha(ver)
    op = dve_ops.DveOp(name, spec, subdim=False, uops_sha=sha)
    dve_ops.OPS.append(op)
    dve_ops._SUB_OPCODE_FOR_NAME[name] = row
    dve_ops.CUSTOM_DVE_SPECS[name] = spec
    return op


def _build_program():
    f32 = mybir.dt.float32
    bf16 = mybir.dt.bfloat16
    cumsum_op = _register_cumsum_op()
    nc = bacc.Bacc(
        trn_type="TRN2", debug=False, num_devices=N_CORES, num_swdge_queues=2
    )
    xp = nc.dram_tensor("xp", [NSB, P, SBW], bf16, kind="ExternalInput").ap()
    xv = nc.dram_tensor("xv", [NVT, P, L], bf16, kind="ExternalInput").ap()
    yp = nc.dram_tensor("yp", [NSB, P, SBW], bf16, kind="ExternalOutput").ap()
    yv = nc.dram_tensor("yv", [NVT, P, L], bf16, kind="ExternalOutput").ap()

    with tile.TileContext(nc) as tc, ExitStack() as ctx:
        const_pool = ctx.enter_context(tc.tile_pool(name="const", bufs=1))
        xin_pool = ctx.enter_context(tc.tile_pool(name="xin", bufs=1))
        xv_pool = ctx.enter_context(tc.tile_pool(name="xv", bufs=1))
        yv_pool = ctx.enter_context(tc.tile_pool(name="yv", bufs=1))
        yout_pool = ctx.enter_context(tc.tile_pool(name="yout", bufs=4))
        small_pool = ctx.enter_context(tc.tile_pool(name="small", bufs=2))
        yps_pool = ctx.enter_context(tc.tile_pool(name="yps", bufs=5, space="PSUM"))
        sps_pool = ctx.enter_context(tc.tile_pool(name="sps", bufs=1, space="PSUM"))
        tps_pool = ctx.enter_context(tc.tile_pool(name="tps", bufs=1, space="PSUM"))

        # ---- constants (gpsimd; runs while the in-DMAs fly) ----
        ut = const_pool.tile([P, P], bf16, name="ut")
        masks.make_upper_triangular(nc, ut[:], 1.0, diag=True)
        z1z = const_pool.tile([P, 2 * GBLK - 1], bf16, name="z1z")
        nc.gpsimd.memset(z1z[:], 0.0)
        nc.gpsimd.memset(z1z[:, GBLK - 1 : GBLK], 1.0)
        rz = const_pool.tile([GBLK, GBLK * P], bf16, name="rz")
        nc.gpsimd.memset(rz[:], 1.0)
        nc.gpsimd.affine_select(
            out=rz[:], in_=rz[:], compare_op=mybir.AluOpType.is_ge,
            fill=0.0, base=0, pattern=[[1, GBLK * P]], channel_multiplier=-P,
        )
        nc.gpsimd.affine_select(
            out=rz[:], in_=rz[:], compare_op=mybir.AluOpType.is_ge,
            fill=0.0, base=P - 1, pattern=[[-1, GBLK * P]], channel_multiplier=P,
        )
        tms = const_pool.tile([GBLK, GBLK], bf16, name="tms")
        masks.make_upper_triangular(nc, tms[:], 1.0, diag=False)
        ones_1x16 = const_pool.tile([1, GBLK], bf16, name="ones_1x16")
        nc.gpsimd.memset(ones_1x16[:], 1.0)
        ones_16x1 = const_pool.tile([GBLK, 1], bf16, name="ones_16x1")
        nc.gpsimd.memset(ones_16x1[:], 1.0)
        one_1x1 = const_pool.tile([1, 1], bf16, name="one_1x1")
        nc.gpsimd.memset(one_1x1[:], 1.0)
        ca0 = const_pool.tile([1, C], bf16, name="ca0")
        nc.gpsimd.memset(ca0[:], 0.0)

        # ---- all in-DMAs up front, in consumption order per ring ----
        # sync ring: PE superblocks interleaved with DVE tile-0 chunks;
        # scalar ring: DVE tile-1/2 chunks.
        xts = {}
        xvc = {}

        def in_xp(s):
            xt = xin_pool.tile([P, SBW], bf16, name=f"xt{s}", tag=f"xt{s}", bufs=1)
            nc.sync.dma_start(out=xt[:], in_=xp[s])
            xts[s] = xt

        def in_xv(t, c, eng):
            xc = xv_pool.tile(
                [P, VCH], bf16, name=f"xv{t}_{c}", tag=f"xv{t}_{c}", bufs=1
            )
            eng.dma_start(out=xc[:], in_=xv[t, :, c * VCH : (c + 1) * VCH])
            xvc[(t, c)] = xc

        in_xp(0)
        in_xv(0, 0, nc.sync)
        in_xp(1)
        in_xv(0, 1, nc.sync)
        in_xp(2)
        in_xv(0, 2, nc.sync)
        in_xp(3)
        in_xv(0, 3, nc.sync)
        for s in range(4, NSB):
            in_xp(s)
        for t in range(1, NVT):
            for c in range(NVC):
                in_xv(t, c, nc.scalar)

        def xsl(blk, nblk=1):
            s, k = blk // SBB, blk % SBB
            return xts[s][:, k * C : (k + nblk) * C]

        # ---- DVE half: chunked custom scans, chained via f32 state col ----
        for t in range(NVT):
            st = yv_pool.tile([P, NVC], f32, name=f"st{t}", tag=f"st{t}", bufs=1)
            for c in range(NVC):
                yc = yv_pool.tile(
                    [P, VCH], bf16, name=f"yv{t}_{c}", tag=f"yv{t}_{c}", bufs=1
                )
                init = 0.0 if c == 0 else st[:, c - 1 : c]
                nc.vector._custom_dve(
                    cumsum_op, out=yc[:], in0=xvc[(t, c)][:], s0=init
                )
                if c < NVC - 1:
                    nc.vector.tensor_copy(st[:, c : c + 1], yc[:, VCH - 1 : VCH])
                (nc.sync if t == 0 else nc.gpsimd).dma_start(
                    out=yv[t, :, c * VCH : (c + 1) * VCH], in_=yc[:]
                )

        # ---- PE half: matmul Blelloch scan ----
        prev_ca = ca0

        def emit_phase1(g):
            sp = sps_pool.tile([GBLK, C], f32, name="sp", tag="sp", bufs=1)
            for i in range(GBLK):
                nc.tensor.matmul(
                    sp[:],
                    z1z[:, GBLK - 1 - i : 2 * GBLK - 1 - i],
                    xsl(GBLK * g + i),
                    start=(i == 0),
                    stop=(i == GBLK - 1),
                )
            sa = small_pool.tile([GBLK, C], bf16, name="sa", tag="sa", bufs=2)
            nc.scalar.copy(sa[:], sp[:])
            return sa

        def emit_carry_math(g, sa):
            nonlocal prev_ca
            ca = prev_ca
            tp = tps_pool.tile([GBLK, C], f32, name="tp", tag="tp", bufs=1)
            nc.tensor.matmul(tp[:], ones_1x16[:], ca[:], start=True, stop=False)
            nc.tensor.matmul(tp[:], tms[:], sa[:], start=False, stop=True)
            tb = small_pool.tile([GBLK, C], bf16, name="tb", tag="tb", bufs=2)
            nc.scalar.copy(tb[:], tp[:])
            if g < NGRP - 1:
                cp = tps_pool.tile([1, C], f32, name="cp", tag="cp", bufs=1)
                nc.tensor.matmul(cp[:], ones_16x1[:], sa[:], start=True, stop=False)
                nc.tensor.matmul(cp[:], one_1x1[:], ca[:], start=False, stop=True)
                nca = small_pool.tile([1, C], bf16, name="nca", tag="nca", bufs=2)
                nc.scalar.copy(nca[:], cp[:])
                prev_ca = nca
            return tb

        def emit_phase3(g, tb):
            # QUADS of adjacent blocks (4 per [128, 512] PSUM bank): 4
            # rz broadcasts (first start=True clears the bank's has_written
            # bits; the rest start=False overwrite their clear quarters),
            # then one 512-col UT matmul accumulates on top. Two quads per
            # cluster share the UT stationary load.
            yt = {}
            for q0 in (0, 8):
                pend = []
                for qi in range(2):
                    ps = yps_pool.tile([P, 4 * C], f32, name="ypp", tag="ypp", bufs=5)
                    for h in range(4):
                        i = q0 + 4 * qi + h
                        blk = GBLK * g + i
                        s, k = blk // SBB, blk % SBB
                        if k == 0 and h == 0:
                            yt[s] = yout_pool.tile(
                                [P, SBW], bf16, name=f"yt{s}", tag="yt", bufs=4
                            )
                        nc.tensor.matmul(
                            ps[:, h * C : (h + 1) * C],
                            rz[:, i * P : (i + 1) * P],
                            tb[:],
                            start=(h == 0),
                            stop=False,
                            skip_group_check=True,
                        )
                    pend.append((q0 + 4 * qi, ps))
                for i0, ps in pend:
                    blk = GBLK * g + i0
                    s, k = blk // SBB, blk % SBB
                    nc.tensor.matmul(
                        ps[:],
                        ut[:],
                        xsl(blk, 4),
                        start=False,
                        stop=True,
                        skip_group_check=True,
                    )
                    nc.scalar.copy(yt[s][:, k * C : (k + 4) * C], ps[:])
                    if k + 4 == SBB:
                        nc.scalar.dma_start(out=yp[s], in_=yt[s][:])

        # schedule: ph_0, ph_1, T_0, p3_0, ph_2, T_1, p3_1, ph_3, T_2, p3_2,
        # T_3, p3_3  (phase1 runs ahead so carries are ready early)
        sas = {}
        tbs = {}
        sas[0] = emit_phase1(0)
        sas[1] = emit_phase1(1)
        tbs[0] = emit_carry_math(0, sas[0])
        emit_phase3(0, tbs[0])
        sas[2] = emit_phase1(2)
        tbs[1] = emit_carry_math(1, sas[1])
        emit_phase3(1, tbs[1])
        sas[3] = emit_phase1(3)
        tbs[2] = emit_carry_math(2, sas[2])
        emit_phase3(2, tbs[2])
        tbs[3] = emit_carry_math(3, sas[3])
        emit_phase3(3, tbs[3])

    nc.compile()
    return nc


def _get_program():
    if "nc" not in _CACHE:
        _CACHE["nc"] = _build_program()
    return _CACHE["nc"]


def _shard(X):
    Xv = X.reshape(B, L, C_FULL)
    shards = []
    for i in range(N_CORES):
        b, h = i // 2, i % 2
        slab = Xv[b, :, h * CH : (h + 1) * CH]          # [L, 512] f32
        pe = slab[:, :C]                                 # [L, 128]
        dv = slab[:, C:]                                 # [L, 384]
        arr_p = (
            pe.reshape(NSB, SBB, P, C).transpose(0, 2, 1, 3).reshape(NSB, P, SBW)
        )
        arr_v = np.ascontiguousarray(dv.T).reshape(NVT, P, L)
        shards.append(
            {
                "xp": np.ascontiguousarray(arr_p).astype(ml_dtypes.bfloat16),
                "xv": arr_v.astype(ml_dtypes.bfloat16),
            }
        )
    return shards


def _unshard(parts):
    out = np.empty((B, L, C_FULL), dtype=np.float32)
    for i in range(N_CORES):
        b, h = i // 2, i % 2
        arr_p = np.asarray(parts[i]["yp"]).astype(np.float32)
        slab_p = (
            arr_p.reshape(NSB, P, SBB, C).transpose(0, 2, 1, 3).reshape(L, C)
        )
        out[b, :, h * CH : h * CH + C] = slab_p
        arr_v = np.asarray(parts[i]["yv"]).astype(np.float32)
        out[b, :, h * CH + C : (h + 1) * CH] = arr_v.reshape(CV, L).T
    return out.reshape(B, L, D, N)


def kernel(X_in, _trace=False, _tmpdir=None, _trace_cores=None):
    X = np.asarray(X_in, dtype=np.float32)
    assert X.shape == (B, L, D, N), X.shape
    nc = _get_program()
    in_maps = _shard(X)
    kwargs = {}
    if _trace:
        kwargs = dict(
            trace=True,
            tmpdir=_tmpdir,
            trace_cores=_trace_cores or list(range(N_CORES)),
        )
    res = run_bass_kernel_spmd(nc, in_maps, core_ids=list(range(N_CORES)), **kwargs)
    out = _unshard(
        [{"yp": res.results[i]["yp"], "yv": res.results[i]["yv"]} for i in range(N_CORES)]
    )
    kernel.last_results = res
    return out


# revision 9
# speedup vs baseline: 1.4453x; 1.0938x over previous
"""Trainium2 Bass kernel: inclusive cumsum along L for X (4, 8192, 32, 32) f32.

Full-DVE design (8 NeuronCores, SPMD), bf16 HBM traffic both ways:
  - Shard: core i gets b = i//2, channel-half = i%2 -> 512 channels x 8192 L,
    host-transposed to [4][128ch][8192L] bf16 (channels on partitions, L on
    the free dim). HBM traffic per core: 8 MiB in + 8 MiB out.
  - Scan: a custom DVE op (ANT_CUMSUM_INIT, registered per-NEFF through the
    documented dve_ops extension point: body = scan(ADD, Src0, init=C0))
    computes the inclusive prefix along the free dim at 1 elem/cycle
    (~1.06 ns/elem measured — 2x the stock tensor_tensor_scan, whose
    feedback-bubble uOp costs 2 cycles/elem) with fp32 ALU state. 16
    chunk-scans of [128, 2048] per core (~2.35 us each, ~38 us total);
    chunks chain through an f32 [P,1] state column (imm0 scalar APs must
    be f32; bf16 state at 3 boundaries/row keeps the error well under
    tolerance). The PE-based Blelloch variant was abandoned: at <=256
    output columns the tensor pipeline is instruction-overhead-bound
    (~330 ns/matmul regardless of width), so its ~160-instruction scan
    never beats the DVE path, and the HAM clock governor adds variance.
  - DMA rings (each HWDGE ring sustains ~210 GB/s; ~420 GB/s aggregate):
    sync ring carries tile-0/1 ins then tile-0/1 outs (8.4 MiB);
    scalar ring carries tile-2/3 ins (4.2 MiB);
    gpsimd SWDGE ring carries tile-2/3 outs (4.2 MiB).
    All ins are issued first, in DVE consumption order, so the scan
    pipeline is DMA-fed ~2.4 us/chunk against ~2.35 us/chunk consumption.
  - Error budget (tolerance 2e-2 * max|out| ~ 9.1): bf16 input quantization
    random-walks to ~0.3; bf16 chunk chaining ~2.7 worst-case; bf16 output
    rounding ~1.8. Measured ~2.4 abs (5e-3 relative).
"""

import numpy as np
import ml_dtypes
from contextlib import ExitStack

import concourse.bass as bass
import concourse.tile as tile
from concourse import bacc, mybir
from concourse.bass_utils import run_bass_kernel_spmd

N_CORES = 8
B, L, D, N = 4, 8192, 32, 32
C_FULL = D * N          # 1024 channels total
CH = C_FULL // 2        # 512 channels per core
P = 128
NVT = CH // P           # 4 DVE tiles of 128 channels
VCH = 2048              # chunk width (cols of L)
NVC = L // VCH          # 4 chunks per tile

_CACHE = {}


def _register_cumsum_op():
    """Per-NEFF custom DVE op: out[p,k] = s0[p] + sum_{j<=k} in0[p,j].
    Appended to dve_ops.OPS with a computed uops_sha (the documented
    per-NEFF DVE-table extension point); runs at 1 elem/cycle."""
    from concourse import dve_ops
    from concourse.dve_spec import Spec, Src0, C0, AluOp, scan, lower
    from concourse.dve_uop import DveOpSpec

    name = "ANT_CUMSUM_INIT"
    for op in dve_ops.OPS:
        if op.name == name:
            return op
    spec = Spec(
        body=scan(AluOp.ADD, Src0, init=C0),
        reference=lambda in0, s0: np.cumsum(in0.astype(np.float32), axis=-1)
        + np.asarray(s0, dtype=np.float32),
    )
    row = dve_ops._CUSTOM_DVE_ROW_BASE + len(dve_ops.OPS)
    sha = {}
    for ver in ("v3", "v4"):
        s = DveOpSpec(name=name, opcode=row, uops=lower(spec, ver=ver), rd1_en=False)
        sha[ver] = s.sha(ver)
    op = dve_ops.DveOp(name, spec, subdim=False, uops_sha=sha)
    dve_ops.OPS.append(op)
    dve_ops._SUB_OPCODE_FOR_NAME[name] = row
    dve_ops.CUSTOM_DVE_SPECS[name] = spec
    return op


def _build_program():
    f32 = mybir.dt.float32
    bf16 = mybir.dt.bfloat16
    cumsum_op = _register_cumsum_op()
    nc = bacc.Bacc(
        trn_type="TRN2", debug=False, num_devices=N_CORES, num_swdge_queues=2
    )
    xv = nc.dram_tensor("xv", [NVT, P, L], bf16, kind="ExternalInput").ap()
    yv = nc.dram_tensor("yv", [NVT, P, L], bf16, kind="ExternalOutput").ap()

    with tile.TileContext(nc) as tc, ExitStack() as ctx:
        xv_pool = ctx.enter_context(tc.tile_pool(name="xv", bufs=1))
        yv_pool = ctx.enter_context(tc.tile_pool(name="yv", bufs=1))

        # ---- all in-DMAs up front, in DVE consumption order per ring ----
        # sync ring: tiles 0-1; scalar ring: tiles 2-3 (land early, consumed
        # late). Interleave emission so both rings start immediately.
        xvc = {}

        def in_xv(t, c, eng):
            xc = xv_pool.tile(
                [P, VCH], bf16, name=f"xv{t}_{c}", tag=f"xv{t}_{c}", bufs=1
            )
            eng.dma_start(out=xc[:], in_=xv[t, :, c * VCH : (c + 1) * VCH])
            xvc[(t, c)] = xc

        for c in range(NVC):
            in_xv(0, c, nc.sync)
            in_xv(2, c, nc.scalar)
        for c in range(NVC):
            in_xv(1, c, nc.sync)
            in_xv(3, c, nc.scalar)

        # ---- chunked custom scans, chained via an f32 state column ----
        for t in range(NVT):
            st = yv_pool.tile([P, NVC], f32, name=f"st{t}", tag=f"st{t}", bufs=1)
            for c in range(NVC):
                yc = yv_pool.tile(
                    [P, VCH], bf16, name=f"yv{t}_{c}", tag=f"yv{t}_{c}", bufs=1
                )
                init = 0.0 if c == 0 else st[:, c - 1 : c]
                nc.vector._custom_dve(
                    cumsum_op, out=yc[:], in0=xvc[(t, c)][:], s0=init
                )
                if c < NVC - 1:
                    nc.vector.tensor_copy(st[:, c : c + 1], yc[:, VCH - 1 : VCH])
                (nc.sync if t < 2 else nc.gpsimd).dma_start(
                    out=yv[t, :, c * VCH : (c + 1) * VCH], in_=yc[:]
                )

    nc.compile()
    return nc


def _get_program():
    if "nc" not in _CACHE:
        _CACHE["nc"] = _build_program()
    return _CACHE["nc"]


def _shard(X):
    Xv = X.reshape(B, L, C_FULL)
    shards = []
    for i in range(N_CORES):
        b, h = i // 2, i % 2
        slab = Xv[b, :, h * CH : (h + 1) * CH]          # [L, 512] f32
        arr_v = np.ascontiguousarray(slab.T).reshape(NVT, P, L)
        shards.append({"xv": arr_v.astype(ml_dtypes.bfloat16)})
    return shards


def _unshard(parts):
    out = np.empty((B, L, C_FULL), dtype=np.float32)
    for i in range(N_CORES):
        b, h = i // 2, i % 2
        arr_v = np.asarray(parts[i]).astype(np.float32)
        out[b, :, h * CH : (h + 1) * CH] = arr_v.reshape(CH, L).T
    return out.reshape(B, L, D, N)


def kernel(X_in, _trace=False, _tmpdir=None, _trace_cores=None):
    X = np.asarray(X_in, dtype=np.float32)
    assert X.shape == (B, L, D, N), X.shape
    nc = _get_program()
    in_maps = _shard(X)
    kwargs = {}
    if _trace:
        kwargs = dict(
            trace=True,
            tmpdir=_tmpdir,
            trace_cores=_trace_cores or list(range(N_CORES)),
        )
    res = run_bass_kernel_spmd(nc, in_maps, core_ids=list(range(N_CORES)), **kwargs)
    out = _unshard([res.results[i]["yv"] for i in range(N_CORES)])
    kernel.last_results = res
    return out
